# revision 17
# baseline (speedup 1.0000x reference)
"""Trainium2 Bass kernel for nn_Attention_18631568130798.

Mixed template/search attention block (Stark-style tracker attention):
  qkv proj -> per-head scores + RPE bias -> template-block softmax ->
  boxmask-weighted factor -> rescaled template->search scores -> softmax ->
  attn @ v -> output proj.

Sharding: data-parallel over batch B=64 across 8 NeuronCores (8 batches/core).
All matmuls run as float32r (full-rate fp32 PE mode, ~1e-4 per-matmul rel err).

Layout strategy (per batch, per head): scores are computed TRANSPOSED,
ST[j, i] = k_j . q_i (j = key token on partitions, i = query token on free
dim), which makes softmax denominators a ones-matmul (PE) and lets
attn @ v produce out^T = (c-partitions, i-free) == exactly the lhsT the
output projection needs. No PE transposes anywhere.
"""

import os
import numpy as np

import concourse.bass as bass
import concourse.mybir as mybir
import concourse.tile as tile
from concourse import bacc
from concourse.bass_utils import run_bass_kernel_spmd

AF = mybir.ActivationFunctionType
F32 = mybir.dt.float32
F32R = mybir.dt.float32r
BF16 = mybir.dt.bfloat16
I16 = mybir.dt.int16

# Problem constants (hardcoded per contract)
B, N, C = 64, 245, 768
H, HD = 12, 64
NT, NS = 49, 196
SCALE = HD ** -0.5
NCORES = 8
NB = B // NCORES              # batches per core
NPAD = 256                    # padded i (query) dim for full-rate f32r matmuls
G_PER = 7504                  # gather indices per gpsimd core group (60032/8)
G_TOT = G_PER * 16 // 16 * 8  # 60032
JCH = [(0, 128), (128, 117)]  # j (key-token) chunks


def _build(nb: int, RPE_SIZE: int, reps: int = 1):
    nc = bacc.Bacc(None, target_bir_lowering=False, name="attn18631")

    xT_d = nc.dram_tensor("xT", [nb, C, N], F32R, kind="ExternalInput")
    bm_d = nc.dram_tensor("bm", [nb, NT, C], F32R, kind="ExternalInput")
    wqkT_d = nc.dram_tensor("wqkT", [C, 2 * C], F32R, kind="ExternalInput")
    wvT_d = nc.dram_tensor("wvT", [C, C], F32R, kind="ExternalInput")
    pwT_d = nc.dram_tensor("pwT", [C, C], F32R, kind="ExternalInput")
    bwT_d = nc.dram_tensor("bwT", [C, H], F32R, kind="ExternalInput")
    projb_d = nc.dram_tensor("projb", [1, C], F32R, kind="ExternalInput")
    boxb_d = nc.dram_tensor("boxb", [1, H], F32R, kind="ExternalInput")
    tab_d = nc.dram_tensor("tab", [128, RPE_SIZE], F32, kind="ExternalInput")
    idx_d = nc.dram_tensor("idx", [128, G_PER // 16], I16, kind="ExternalInput")
    out_d = nc.dram_tensor("out", [nb, N, C], F32, kind="ExternalOutput")
    G_d = nc.dram_tensor("G", [H, 8 * G_PER], F32)  # gather bounce

    with tile.TileContext(nc) as tc:
        res = tc.alloc_tile_pool(name="res", bufs=1)       # resident singles
        xp = tc.alloc_tile_pool(name="xp", bufs=1)         # x pair + gather temps
        qkp = tc.alloc_tile_pool(name="qkp", bufs=2)       # per-batch qkT
        ep = tc.alloc_tile_pool(name="ep", bufs=1)         # E tiles
        sm = tc.alloc_tile_pool(name="sm", bufs=1)         # misc small per-batch
        ps = tc.alloc_tile_pool(name="ps", bufs=3, space="PSUM")
        po = tc.alloc_tile_pool(name="po", bufs=3, space="PSUM")
        pt = tc.alloc_tile_pool(name="pt", bufs=2, space="PSUM")

        # ---------------- preamble: constants + weights ----------------
        cst = res.tile([128, 4], F32, tag="cst")
        nc.vector.memset(cst[:, 0:1], 1.0)
        nc.vector.memset(cst[:, 1:2], float(N - NT))
        nc.vector.memset(cst[:, 2:3], 0.0)
        c196 = cst[0:1, 1:2]
        ones = res.tile([1, 512], F32R, tag="ones")
        nc.scalar.copy(ones, cst[0:1, 0:1].to_broadcast((1, 512)))
        onescol = res.tile([128, 1], F32R, tag="onescol")
        nc.scalar.copy(onescol, cst[:, 0:1])
        # row masks for the template->search rescale: rows j<NT keep factor 1.0
        maskf = res.tile([1, 128], F32, tag="maskf")
        nc.vector.memset(maskf, 0.0)
        nc.vector.memset(maskf[0:1, NT:128], 1.0)
        imaskf = res.tile([1, 128], F32, tag="imaskf")
        nc.vector.memset(imaskf, 1.0)
        nc.vector.memset(imaskf[0:1, NT:128], 0.0)
        maskts = res.tile([1, 128], F32R, tag="maskts")
        nc.scalar.copy(maskts, maskf[:])
        imaskts = res.tile([1, 128], F32R, tag="imaskts")
        nc.scalar.copy(imaskts, imaskf[:])

        wqkT = res.tile([128, 6, 2 * C], F32R, tag="wqkT")
        nc.sync.dma_start(out=wqkT, in_=wqkT_d.rearrange("(cc p) o -> p cc o", p=128))
        wvT = res.tile([128, 6, 2, 384], F32R, tag="wvT")
        nc.sync.dma_start(out=wvT, in_=wvT_d.rearrange("(cc p) (hf o) -> p cc hf o", p=128, hf=2))
        pwT = res.tile([128, 6, 2, 384], F32R, tag="pwT")
        nc.sync.dma_start(out=pwT, in_=pwT_d.rearrange("(cc p) (hf o) -> p cc hf o", p=128, hf=2))
        bwT = res.tile([128, 6, H], F32R, tag="bwT")
        nc.sync.dma_start(out=bwT, in_=bwT_d.rearrange("(cc p) h -> p cc h", p=128))
        projb = res.tile([1, C], F32R, tag="projb")
        nc.sync.dma_start(out=projb, in_=projb_d[:])
        boxb = res.tile([1, H], F32R, tag="boxb")
        nc.sync.dma_start(out=boxb, in_=boxb_d[:])
        # zero-padded per-head lhsT tiles: head h occupies column half h%2, the
        # other half stays zero so head pairs can accumulate into one base-0 psum
        v_ext = res.tile([128, 2, H, 2, HD], F32R, tag="vext")
        nc.vector.memset(v_ext.bitcast(F32), 0.0)
        bm_ext = res.tile([NT, H, 2, HD], F32R, tag="bmext")
        nc.vector.memset(bm_ext.bitcast(F32), 0.0)

        # ---------------- preamble: RPE bias gather -> biasT ----------------
        tab = xp.tile([128, RPE_SIZE], F32, tag="xpair")
        nc.sync.dma_start(out=tab, in_=tab_d[:])
        idxt = res.tile([128, G_PER // 16], I16, tag="idx")
        nc.sync.dma_start(out=idxt, in_=idx_d[:])
        # gather in 4 column-chunks to keep SBUF small; chunk sizes mult of 16
        gchunks = [1872, 1872, 1872, 1888]
        goff = 0
        for gsz in gchunks:
            gt = xp.tile([128, 1888], F32, tag="xpair2")
            nc.gpsimd.ap_gather(
                out_ap=gt[:, 0:gsz], in_ap=tab[:], idxs_ap=idxt[:, goff // 16:(goff + gsz) // 16],
                channels=128, num_elems=RPE_SIZE, d=1, num_idxs=gsz,
            )
            for g in range(8):
                nc.sync.dma_start(out=G_d[0:H, g * G_PER + goff: g * G_PER + goff + gsz],
                                  in_=gt[16 * g:16 * g + H, 0:gsz])
            goff += gsz

        # biasT[ch][j, h, i] = rpe_table[h, rpe_index[i, 128*ch + j]]  (bf16)
        biasT = []
        for ch, (j0, jw) in enumerate(JCH):
            bt = res.tile([128, H, N], BF16, tag=f"biasT{ch}")
            biasT.append(bt)
            for h in range(H):
                tmpf = xp.tile([128, N], F32, tag="xpair2")
                nc.sync.dma_start(
                    out=tmpf[0:jw, :],
                    in_=G_d[h, j0 * N: (j0 + jw) * N].rearrange("(j i) -> j i", i=N))
                nc.scalar.copy(bt[0:jw, h, :], tmpf[0:jw, :])

        # ---------------- main per-batch-pair loop ----------------
        npairs = ((nb + 1) // 2) * reps
        for pair0 in range(npairs):
            pair = pair0 % ((nb + 1) // 2)
            bpair = [min(2 * pair, nb - 1), min(2 * pair + 1, nb - 1)]
            dup = bpair[0] == bpair[1]
            if dup:
                bpair = bpair[:1]
            nbp = 2
            xpair = xp.tile([128, 6, 2, N], F32R, tag="xpair")
            for t in range(2):
                b = bpair[min(t, len(bpair) - 1)]
                nc.sync.dma_start(out=xpair[:, :, t, :],
                                  in_=xT_d[b].rearrange("(cc p) i -> p cc i", p=128))

            # qk projection (transposed):  qkT[b][p, m, i]
            # o-chunk m holds "qkh" blocks 2m (partitions 0:64) and 2m+1 (64:128);
            # qkh 0..11 = q heads (pre-scaled), 12..23 = k heads.
            qkT = [qkp.tile([128, H, NPAD], F32R, tag="qkT", name=f"qkT{t}") for t in range(len(bpair))]
            for m in range(12):
                p_qk = ps.tile([128, 2, N], F32, tag="mm")
                for cc in range(6):
                    nc.tensor.matmul(p_qk[:], wqkT[:, cc, 128 * m:128 * m + 128],
                                     xpair[:, cc, :, :],
                                     start=(cc == 0), stop=(cc == 5))
                for t in range(len(bpair)):
                    nc.scalar.copy(qkT[t][:, m, 0:N], p_qk[:, t, :])

            for t, b in enumerate(bpair):
                qk = qkT[t]
                # ---- v projection (row-major) into zero-padded halves
                for ic, (i0, iw) in enumerate(JCH):
                    for hf in range(2):
                        p_v = ps.tile([128, 384], F32, tag="mm")
                        for cc in range(6):
                            nc.tensor.matmul(p_v[0:iw, :], xpair[:, cc, t, i0:i0 + iw],
                                             wvT[:, cc, hf, :],
                                             start=(cc == 0), stop=(cc == 5))
                        pv6 = p_v.rearrange("p (r d) -> p r d", r=6)
                        nc.scalar.copy(v_ext[0:iw, ic, 6 * hf:6 * hf + 6:2, 0, :],
                                       pv6[0:iw, 0:6:2, :])
                        nc.scalar.copy(v_ext[0:iw, ic, 6 * hf + 1:6 * hf + 6:2, 1, :],
                                       pv6[0:iw, 1:6:2, :])

                # ---- scores + exp:  E[ch][j, h, i] = exp(k.q*scale + bias)
                E = [ep.tile([128, H, NPAD], F32R, tag=f"E{ch}", name=f"E{ch}") for ch in range(2)]
                A_ts = [sm.tile([128, H, NT], F32R, tag=f"Ats{ch}", name=f"Ats{ch}") for ch in range(2)]
                for h in range(H):
                    s, tq, tk = h % 2, h // 2, 6 + h // 2
                    for ch, (j0, jw) in enumerate(JCH):
                        p_st = ps.tile([128, NPAD], F32, tag="mm")
                        nc.tensor.matmul(p_st[0:jw, :],
                                         qk[64 * s:64 * s + 64, tk, j0:j0 + jw],
                                         qk[64 * s:64 * s + 64, tq, :],
                                         start=True, stop=True)
                        # A = scores + bias (written into E tile, exp'd in place)
                        nc.vector.tensor_add(E[ch][0:jw, h, 0:N], p_st[0:jw, 0:N],
                                             biasT[ch][0:jw, h, :])
                        if ch == 0:
                            nc.vector.tensor_copy(A_ts[0][:, h, :], E[0][:, h, 0:NT])
                        else:
                            nc.vector.tensor_copy(A_ts[1][0:117, h, :], E[1][0:117, h, 0:NT])
                        nc.scalar.activation(E[ch][0:jw, h, 0:N], E[ch][0:jw, h, 0:N], AF.Exp)
                    # zero the padded i columns (needed: E is matmul rhs)
                    for ch, (j0, jw) in enumerate(JCH):
                        nc.vector.memset(E[ch][0:jw, h, N:NPAD].bitcast(F32), 0.0)

                # ---- template path
                bm_sb = xp.tile([NT, H, HD], F32R, tag="xpair2")
                nc.sync.dma_start(out=bm_sb, in_=bm_d[b].rearrange("j (h d) -> j h d", h=H))
                nc.scalar.copy(bm_ext[0:NT, 0:H:2, 0, :], bm_sb[0:NT, 0:H:2, :])
                nc.scalar.copy(bm_ext[0:NT, 1:H:2, 1, :], bm_sb[0:NT, 1:H:2, :])
                # template denominators (+196 for the zeroed row tail), reciprocal,
                # broadcast across partitions — before the per-head matmuls so the
                # normalize can consume each OtT psum immediately.
                tdrow = sm.tile([1, 2, 6 * NT], F32, tag="tdrow")
                for hf in range(2):
                    p_td = pt.tile([1, 512], F32, tag="tiny")
                    nc.tensor.matmul(p_td[0:1, 0:6 * NT], onescol[0:NT, 0:1].bitcast(F32),
                                     E[0][0:NT, 6 * hf:6 * hf + 6, 0:NT].bitcast(F32),
                                     start=True, stop=True)
                    nc.scalar.activation(tdrow[0:1, hf, :], p_td[0:1, 0:6 * NT],
                                         AF.Identity, bias=c196[0:1, 0:1])
                rT = sm.tile([1, 2 * 6 * NT], F32R, tag="rT")
                with nc.allow_low_precision("f32r reciprocal feeds f32r matmul"):
                    nc.vector.reciprocal(rT[0:1, :], tdrow[0:1, :, :])
                rb_sb = sm.tile([128, 2, 6 * NT], F32, tag="bcast")
                for hf in range(2):
                    p_rb = ps.tile([128, 6 * NT], F32, tag="mm")
                    nc.tensor.matmul(p_rb[:], ones[0:1, 0:128],
                                     rT[0:1, 6 * NT * hf:6 * NT * (hf + 1)],
                                     start=True, stop=True)
                    nc.scalar.copy(rb_sb[:, hf, :], p_rb[:])
                # normalized template out (t_vec^T) stacked over heads -> c chunks
                otn = sm.tile([128, 6, NT + 1], F32R, tag="otn")
                for tq in range(6):
                    p_ot = po.tile([128, NT + 1], F32, tag="ot")
                    for s in range(2):
                        h = 2 * tq + s
                        nc.tensor.matmul(p_ot[:], bm_ext[0:NT, h, :, :],
                                         E[0][0:NT, h, 0:NT + 1],
                                         start=(s == 0), stop=(s == 1))
                    for s in range(2):
                        h = 2 * tq + s
                        nc.vector.tensor_mul(
                            otn[64 * s:64 * s + 64, tq, 0:NT],
                            p_ot[64 * s:64 * s + 64, 0:NT],
                            rb_sb[64 * s:64 * s + 64, h // 6, NT * (h % 6):NT * (h % 6) + NT])
                # factorT[h, i] then broadcast over partitions
                p_f = pt.tile([H, NT + 1], F32, tag="tiny")
                for cc in range(6):
                    nc.tensor.matmul(p_f[:], bwT[:, cc, :], otn[:, cc, :],
                                     start=(cc == 0), stop=False)
                nc.tensor.matmul(p_f[:], boxb[0:1, :], ones[0:1, 0:NT + 1],
                                 start=False, stop=True)
                facT = sm.tile([H, NT], F32R, tag="facT")
                nc.scalar.copy(facT[:], p_f[:, 0:NT])
                facflat = sm.tile([1, H * NT], F32R, tag="facflat")
                nc.sync.dma_start(out=facflat[0:1].rearrange("o (h i) -> o h i", h=H),
                                  in_=facT[0:H, 0:NT])
                fb0 = sm.tile([128, 2, 6 * NT], F32, tag="bcast")
                fb1 = sm.tile([128, 2, 6 * NT], F32, tag="fb1")
                for hf in range(2):
                    fsl = facflat[0:1, 6 * NT * hf:6 * NT * (hf + 1)]
                    p_fb = ps.tile([128, 6 * NT], F32, tag="mm")
                    nc.tensor.matmul(p_fb[:], maskts[0:1, :], fsl, start=True, stop=False)
                    nc.tensor.matmul(p_fb[:], imaskts[0:1, :], ones[0:1, 0:6 * NT],
                                     start=False, stop=True)
                    nc.scalar.copy(fb0[:, hf, :], p_fb[:])
                    p_fb1 = ps.tile([128, 6 * NT], F32, tag="mm")
                    nc.tensor.matmul(p_fb1[:], ones[0:1, 0:128], fsl, start=True, stop=True)
                    nc.scalar.copy(fb1[:, hf, :], p_fb1[:])
                # rescale template-row -> search-col scores:  E = exp(A * factor)
                # (chunk0 rows j<NT multiply by 1.0 -> bit-identical recompute)
                for h in range(H):
                    hf, o = h // 6, NT * (h % 6)
                    nc.vector.tensor_mul(A_ts[0][:, h, :], A_ts[0][:, h, :],
                                         fb0[:, hf, o:o + NT])
                    nc.scalar.activation(E[0][:, h, 0:NT], A_ts[0][:, h, :], AF.Exp)
                    nc.vector.tensor_mul(A_ts[1][0:117, h, :], A_ts[1][0:117, h, :],
                                         fb1[0:117, hf, o:o + NT])
                    nc.scalar.activation(E[1][0:117, h, 0:NT], A_ts[1][0:117, h, :], AF.Exp)

                # ---- main softmax denominators + attn @ v (head pairs)
                OTn = sm.tile([128, 6, N], F32R, tag="OTn")
                for tq in range(6):
                    p_d = pt.tile([1, 512], F32, tag="tiny")
                    nc.tensor.matmul(p_d[:], onescol[:, 0:1],
                                     E[0][:, 2 * tq:2 * tq + 2, :], start=True, stop=False)
                    nc.tensor.matmul(p_d[:], onescol[0:117, 0:1],
                                     E[1][0:117, 2 * tq:2 * tq + 2, :], start=False, stop=True)
                    rec = sm.tile([1, 2, NPAD], F32R, tag="rec")
                    with nc.allow_low_precision("f32r reciprocal feeds f32r matmul"):
                        nc.vector.reciprocal(rec[0:1, :, 0:N], p_d[0:1].rearrange("o (u i) -> o u i", u=2)[:, :, 0:N])
                    p_r2 = ps.tile([128, 512], F32, tag="mm")
                    nc.tensor.matmul(p_r2[:], ones[0:1, 0:128], rec[0:1, :, :],
                                     start=True, stop=True)
                    rb2 = sm.tile([128, 2, NPAD], F32, tag="rb2")
                    nc.scalar.copy(rb2[:], p_r2[:])
                    p_o = po.tile([128, NPAD], F32, tag="ot")
                    for s in range(2):
                        h = 2 * tq + s
                        for ch, (j0, jw) in enumerate(JCH):
                            nc.tensor.matmul(p_o[:], v_ext[0:jw, ch, h, :, :],
                                             E[ch][0:jw, h, :],
                                             start=(s == 0 and ch == 0),
                                             stop=(s == 1 and ch == 1))
                    for s in range(2):
                        nc.vector.tensor_mul(OTn[64 * s:64 * s + 64, tq, :],
                                             p_o[64 * s:64 * s + 64, 0:N],
                                             rb2[64 * s:64 * s + 64, s, 0:N])

                # ---- output projection, DMA straight from PSUM
                for ic, (i0, iw) in enumerate(JCH):
                    y_sb = sm.tile([128, C], F32, tag="y")
                    for hf in range(2):
                        p_y = ps.tile([128, 384], F32, tag="mm")
                        for cc in range(6):
                            nc.tensor.matmul(p_y[0:iw, :], OTn[:, cc, i0:i0 + iw],
                                             pwT[:, cc, hf, :],
                                             start=(cc == 0), stop=False)
                        nc.tensor.matmul(p_y[0:iw, :], ones[0:1, 0:iw],
                                         projb[0:1, 384 * hf:384 * hf + 384],
                                         start=False, stop=True)
                        nc.scalar.copy(y_sb[0:iw, 384 * hf:384 * hf + 384], p_y[0:iw, :])
                    nc.sync.dma_start(out=out_d[b, i0:i0 + iw, :], in_=y_sb[0:iw, :])

        for p in (pt, po, ps, sm, ep, qkp, xp, res):
            p.release()

    nc.finalize()
    return nc


_CACHE = {}


def _get_nc(nb, rpe_size, reps=1):
    key = (nb, rpe_size, reps)
    if key not in _CACHE:
        _CACHE[key] = _build(nb, rpe_size, reps)
    return _CACHE[key]


def _prep_core_inputs(x, boxmask_vec, qkv_w, qkv_b, proj_w, proj_b, box_w, box_b,
                      rpe_table, rpe_index):
    RPE_SIZE = rpe_table.shape[1]
    assert RPE_SIZE <= 8192 and rpe_index.max() < RPE_SIZE
    """Host-side input prep shared across cores (weights) — returns dict template."""
    assert np.allclose(qkv_b, 0.0), "kernel assumes qkv_b == 0 (spec fill: zeros)"
    Wq = qkv_w[:C] * np.float32(SCALE)
    Wk = qkv_w[C:2 * C]
    Wv = qkv_w[2 * C:]
    wqkT = np.ascontiguousarray(np.concatenate([Wq, Wk], 0).T)       # (768, 1536)
    wvT = np.ascontiguousarray(Wv.T)                                  # (768, 768)
    pwT = np.ascontiguousarray(proj_w.T)                              # (768, 768)
    bwT = np.ascontiguousarray(box_w.T)                               # (768, 12)
    tab = np.zeros((128, RPE_SIZE), np.float32)
    tab[:] = np.tile(np.concatenate([rpe_table, np.zeros((4, RPE_SIZE), np.float32)], 0), (8, 1))
    flat = np.zeros(8 * G_PER, np.int64)
    flat[:N * N] = rpe_index.T.reshape(-1)                            # j-major
    idx = np.zeros((128, G_PER // 16), np.int16)
    for g in range(8):
        blk = flat[g * G_PER:(g + 1) * G_PER].reshape(G_PER // 16, 16)  # [s, r]
        idx[16 * g:16 * (g + 1), :] = blk.T.astype(np.int16)
    return {
        "wqkT": wqkT, "wvT": wvT, "pwT": pwT, "bwT": bwT,
        "projb": np.ascontiguousarray(proj_b[None, :].astype(np.float32)),
        "boxb": np.ascontiguousarray(box_b[None, :].astype(np.float32)),
        "tab": tab, "idx": idx,
    }


def kernel(x, boxmask_vec, qkv_w, qkv_b, proj_w, proj_b, box_w, box_b,
           rpe_table, rpe_index, lens_t, _nb=NB, _trace=False, _reps=1):
    x = np.asarray(x, np.float32)
    boxmask_vec = np.asarray(boxmask_vec, np.float32)
    qkv_w = np.asarray(qkv_w, np.float32)
    qkv_b = np.asarray(qkv_b, np.float32)
    proj_w = np.asarray(proj_w, np.float32)
    proj_b = np.asarray(proj_b, np.float32)
    box_w = np.asarray(box_w, np.float32)
    box_b = np.asarray(box_b, np.float32)
    rpe_table = np.asarray(rpe_table, np.float32)
    rpe_index = np.asarray(rpe_index, np.int32)
    assert int(lens_t) == NT and x.shape == (B, N, C)

    shared = _prep_core_inputs(x, boxmask_vec, qkv_w, qkv_b, proj_w, proj_b,
                               box_w, box_b, rpe_table, rpe_index)
    nb = _nb
    nc = _get_nc(nb, rpe_table.shape[1], _reps)
    in_maps = []
    for c in range(NCORES):
        bs = [min(c * nb + i, B - 1) for i in range(nb)]
        m = dict(shared)
        m["xT"] = np.ascontiguousarray(x[bs].transpose(0, 2, 1))
        m["bm"] = np.ascontiguousarray(boxmask_vec[bs])
        in_maps.append(m)
    res = run_bass_kernel_spmd(nc, in_maps, list(range(NCORES)),
                               trace=_trace or bool(os.environ.get("BASS_TRACE")))
    out = np.empty((NCORES * nb, N, C), np.float32)
    for c in range(NCORES):
        out[c * nb:(c + 1) * nb] = res.results[c]["out"]
    if _trace:
        kernel._last = res
    return out[:B] if nb == NB else out
